# revision 20
# baseline (speedup 1.0000x reference)
"""Trainium2 Bass kernel for nn_BESNumEigen3qubitModel (v2).

Same math reduction as v1 (eigenvalues of rho, pt_a(rho), pt_c(rho) per batch
element drive the whole loss), with a faster device algorithm:

  - column-major float layout (f = 64*h + 8*j + i for re/im h of entry (i,j))
    so matrix columns are unit-stride runs -> DVE 2x fp16 mode applies
  - matrices stored fp16, diagonals in a separate f32 tensor
  - 2 full Jacobi sweeps (all 3*32 matrices) + 1 rho-only sweep + a one-shot
    second-order diagonal correction for rho (replaces a 4th sweep)
  - per-round batched rotation-parameter chain (all 4 XOR pairs at once via
    affine access patterns), per-pair column rotations + Hermitian row restore
"""

import numpy as np

D = 8
BATCH = 32768
NCORES = 8
PER_CORE = BATCH // NCORES       # 4096
NTILES = PER_CORE // 128         # 32 rho matrices per partition
NM = 3 * NTILES                  # 96 matrices per partition (type-major)

_f32 = np.float32


# ---------------------------------------------------------------- host prep --

def _gellmann_basis(d):
    mats = []
    for j in range(d):
        for k in range(j + 1, d):
            m = np.zeros((d, d), np.complex128); m[j, k] = 1; m[k, j] = 1
            mats.append(m)
    for j in range(d):
        for k in range(j + 1, d):
            m = np.zeros((d, d), np.complex128); m[j, k] = -1j; m[k, j] = 1j
            mats.append(m)
    for l in range(1, d):
        m = np.zeros((d, d), np.complex128)
        m[np.arange(l), np.arange(l)] = 1
        m[l, l] = -l
        mats.append(np.sqrt(2.0 / (l * (l + 1))) * m)
    return np.stack(mats)


def _pt(m, kind):
    if kind == 'a':
        return np.swapaxes(m.reshape(2, 4, 2, 4), 1, 3).reshape(8, 8)
    return np.swapaxes(m.reshape(4, 2, 4, 2), 1, 3).reshape(8, 8)


def _build_maps():
    """[64, 3*128] f32 map (vec,1) -> col-major floats of rho/pt_a/pt_c,
    and [64, 3*8] diagonal map."""
    G = _gellmann_basis(D)
    basis = list(G) + [np.eye(D) / D]
    M3 = np.zeros((64, 3 * 128), np.float64)
    MD = np.zeros((64, 3 * 8), np.float64)
    for k, A in enumerate(basis):
        for t, At in enumerate((A, _pt(A, 'a'), _pt(A, 'c'))):
            # col-major: f = 64*h + 8*j + i
            M3[k, t * 128:t * 128 + 64] = At.real.T.reshape(-1)
            M3[k, t * 128 + 64:t * 128 + 128] = At.imag.T.reshape(-1)
            MD[k, t * 8:(t + 1) * 8] = np.diagonal(At).real
    return M3.astype(_f32), MD.astype(_f32)


_MAPS = None


def _host_prep(rho_vec):
    global _MAPS
    if _MAPS is None:
        _MAPS = _build_maps()
    M3, MD = _MAPS
    vec = rho_vec.astype(np.float64)
    vec = vec / np.linalg.norm(vec, axis=-1, keepdims=True)
    vec_aug = np.concatenate(
        [vec.astype(_f32), np.ones((vec.shape[0], 1), _f32)], axis=1)
    flat = (vec_aug @ M3).astype(np.float16)               # [B, 384]
    dflat = vec_aug @ MD                                   # [B, 24] f32
    arr = flat.reshape(NCORES, NTILES, 128, 3, 128)
    darr = dflat.reshape(NCORES, NTILES, 128, 3, 8)
    ins = []
    for c in range(NCORES):
        m = np.ascontiguousarray(
            arr[c].transpose(1, 2, 0, 3).reshape(128, NM * 128))
        dg = np.ascontiguousarray(
            darr[c].transpose(1, 2, 0, 3).reshape(128, NM * 8).astype(_f32))
        ins.append({"mats": m, "diag": dg})
    return ins


# ------------------------------------------------------------ device kernel --

def _msb(r):
    return 4 if r >= 4 else (2 if r >= 2 else 1)


# Batcher odd-even mergesort network for 8 elements (19 comparators)
_CE8 = [(0, 1), (2, 3), (4, 5), (6, 7), (0, 2), (1, 3), (4, 6), (5, 7),
        (1, 2), (5, 6), (0, 4), (1, 5), (2, 6), (3, 7), (2, 4), (3, 5),
        (1, 2), (3, 4), (5, 6)]


def _build_program(k0, k1):
    import concourse.bass as bass
    import concourse.bacc as bacc
    import concourse.mybir as mybir
    from concourse.tile import TileContext
    from contextlib import ExitStack

    f32 = mybir.dt.float32
    f16 = mybir.dt.float16
    ALU = mybir.AluOpType
    ACT = mybir.ActivationFunctionType

    nc = bacc.Bacc("TRN2")
    mats_d = nc.dram_tensor("mats", [128, NM * 128], f16, kind="ExternalInput")
    diag_d = nc.dram_tensor("diag", [128, NM * 8], f32, kind="ExternalInput")
    out_d = nc.dram_tensor("out", [128, NTILES], f32, kind="ExternalOutput")

    with ExitStack() as ctx:
        tc = ctx.enter_context(TileContext(nc))
        main = ctx.enter_context(tc.tile_pool(name="main", bufs=1))
        pp = ctx.enter_context(tc.tile_pool(name="pp", bufs=3))
        ep = ctx.enter_context(tc.tile_pool(name="ep", bufs=2))
        cp = ctx.enter_context(tc.tile_pool(name="cp", bufs=2))

        A = main.tile([128, NM, 128], f16, name="A")
        Dg = main.tile([128, NM, 8], f32, name="Dg")
        for ch in range(8):
            nc.sync.dma_start(
                out=A[:, ch * 12:(ch + 1) * 12, :],
                in_=mats_d[:, ch * 12 * 128:(ch + 1) * 12 * 128])
        nc.sync.dma_start(out=Dg[:, :, :], in_=diag_d[:, :])

        eps30 = main.tile([128, 1], f32, name="eps30")
        nc.vector.memset(eps30[:], 1e-30)
        eps35 = main.tile([128, 1], f32, name="eps35")
        nc.vector.memset(eps35[:], 1e-35)

        Aap = A[:]
        pdim = list(Aap.ap[0])
        Dap = Dg[:]
        dpdim = list(Dap.ap[0])

        def aAP(off, dims, M, m0=0):
            return bass.AP(tensor=Aap.tensor,
                           offset=Aap.offset + off + m0 * 128,
                           ap=[list(pdim), [128, M], *[list(d) for d in dims]])

        def dAP(off, dims, M):
            return bass.AP(tensor=Dap.tensor, offset=Dap.offset + off,
                           ap=[list(dpdim), [8, M], *[list(d) for d in dims]])

        def tAP(t, off, dims, M, m0=0):
            # custom inner-dim view of a [128, NM, ...] tile's [m0:m0+M] slice
            tap = t[:]
            return bass.AP(tensor=tap.tensor,
                           offset=tap.offset + off + m0 * tap.ap[1][0],
                           ap=[list(tap.ap[0]), [tap.ap[1][0], M],
                               *[list(d) for d in dims]])

        negone = main.tile([128, 16], f16, name="negone")
        nc.vector.memset(negone[:], -1.0)

        def negbc(Mx):
            nap = negone[:]
            return bass.AP(tensor=nap.tensor, offset=nap.offset,
                           ap=[list(nap.ap[0]), [0, Mx], [0, 2], [1, 8]])

        TT = nc.vector.tensor_tensor
        GT = nc.gpsimd.tensor_tensor
        STT = nc.vector.scalar_tensor_tensor
        GSTT = nc.gpsimd.scalar_tensor_tensor

        # ---------------- one Jacobi round -------------------------------
        def emit_params(r, M, Dv_off, A_off, Mmats, tbdst=None, pre=""):
            """Rotation params for round r over M matrices.

            Returns (csm_t, srp_t, sip_t) tiles [128, Mmats, 4] f32 holding
            per-pair c / sr / si. If tbdst is None, Dg is updated in place
            (dpp/dqq); else tb is accumulated into tbdst AP pair positions.
            """
            hi = _msb(r)
            b1, b2 = [b for b in (1, 2, 4) if b != hi][::-1]  # b1 outer (larger)
            sg_ = lambda b: -b if (r & b) else b
            app = dAP(Dv_off, [[b1, 2], [b2, 2]], M)
            aqq = dAP(Dv_off + r, [[sg_(b1), 2], [sg_(b2), 2]], M)
            sX = [b + 8 * sg_(b) for b in (b1, b2)]
            X = aAP(A_off + 8 * r, [[sX[0], 2], [sX[1], 2]], M)
            Y = aAP(A_off + 8 * r + 64, [[sX[0], 2], [sX[1], 2]], M)

            tiles = {}

            def pt_(tag):
                t = pp.tile([128, Mmats, 4], f32, tag=pre + tag, name=pre + tag)
                tiles[tag] = t
                return (tAP(t, 0, [[2, 2], [1, 2]], M),   # [M, 2, 2] view
                        tAP(t, 0, [[1, 4]], M))           # flat [M, 4] view

            xx, xxf = pt_("xx")
            yy, yyf = pt_("yy")
            m2, m2f = pt_("m2")
            g, gf = pt_("g")
            g2, g2f = pt_("g2")
            s2, s2f = pt_("s2")
            rs, rsf = pt_("rs")
            h, hf = pt_("h")
            ag, agf = pt_("ag")
            den, denf = pt_("den")
            T, Tf = pt_("T")
            sgn, sgnf = pt_("sgn")
            v, vf = pt_("v")
            tb, tbf = pt_("tb")

            lat = True
            MT = TT if lat else GT
            nc.scalar.activation(xx, X, ACT.Square)
            nc.scalar.activation(yy, Y, ACT.Square)
            MT(m2f, xxf, yyf, ALU.add)
            GT(g, app, aqq, ALU.subtract)
            nc.scalar.activation(g2f, gf, ACT.Square)
            if lat:
                STT(s2f, m2f, 4.0, g2f, ALU.mult, ALU.add)
            else:
                GT(s2f, m2f, m2f, ALU.add)
                GT(s2f, s2f, s2f, ALU.add)
                GT(s2f, s2f, g2f, ALU.add)
            nc.scalar.activation(rsf, s2f, ACT.Abs_reciprocal_sqrt,
                                 bias=eps30[:])
            nc.scalar.activation(agf, gf, ACT.Abs)
            MT(hf, s2f, rsf, ALU.mult)
            MT(denf, agf, hf, ALU.add)
            if lat:
                nc.vector.reciprocal(Tf, denf)
            else:
                GT(Tf, denf, denf, ALU.mult)
                nc.scalar.activation(Tf, Tf, ACT.Abs_reciprocal_sqrt,
                                     bias=eps30[:])   # T = 1/den
            nc.scalar.sign(sgnf, gf, bias=eps35[:])
            GT(vf, m2f, Tf, ALU.mult)
            GT(vf, vf, vf, ALU.add)              # v = 2*m2*T
            GT(tbf, vf, sgnf, ALU.mult)          # tb = 2*m2*T*sg (full)

            if tbdst is None:
                GT(app, app, tb, ALU.add)        # Dg[p] += tb
                GT(aqq, aqq, tb, ALU.subtract)   # Dg[q] -= tb
                T2, T2f = pt_("T2")
                t2, t2f = pt_("t2")
                csm, csmf = pt_("csm")
                u2, u2f = pt_("u2")
                urb2, urb2f = pt_("urb2")
                srp, srpf = pt_("srp")
                sip, sipf = pt_("sip")
                nc.scalar.activation(T2f, Tf, ACT.Square)
                STT(t2f, T2f, 4.0, m2f, ALU.mult, ALU.mult)
                nc.scalar.activation(csmf, t2f, ACT.Abs_reciprocal_sqrt,
                                     bias=1.0)
                GT(u2f, Tf, sgnf, ALU.mult)
                STT(urb2f, u2f, 2.0, csmf, ALU.mult, ALU.mult)
                TT(srp, urb2, X, ALU.mult)
                TT(sip, urb2, Y, ALU.mult)
                return tiles["csm"], tiles["srp"], tiles["sip"]
            else:
                tbp = tbdst(0, [[b1, 2], [b2, 2]])
                tbq = tbdst(r, [[sg_(b1), 2], [sg_(b2), 2]])
                GT(tbp, tbp, tb, ALU.add)
                GT(tbq, tbq, tb, ALU.subtract)
                return None

        def emit_round(r, M, Mc):
            """Round r: params over M matrices, col updates over Mc."""
            hi = _msb(r)
            b1, b2 = [b for b in (1, 2, 4) if b != hi][::-1]
            sg_ = lambda b: -b if (r & b) else b
            pairs = [(a, a ^ r) for a in (0, b2, b1, b1 + b2)]

            csm_t, srp_t, sip_t = emit_params(r, M, 0, 0, NM)

            # expansions over the 8-run (packed last dim for DVE 2x)
            c8t = ep.tile([128, NM, 4, 8], f16, tag="c8", name="c8")
            sr8t = ep.tile([128, NM, 4, 8], f16, tag="sr8", name="sr8")
            si8t = ep.tile([128, NM, 4, 8], f16, tag="si8", name="si8")

            def bc8(t, Mx):
                return tAP(t, 0, [[1, 4], [0, 8]], Mx)

            nc.scalar.copy(c8t[:][:, 0:Mc], bc8(csm_t, Mc))
            nc.vector.tensor_copy(sr8t[:][:, 0:Mc], bc8(srp_t, Mc))
            nc.scalar.copy(si8t[:][:, 0:Mc], bc8(sip_t, Mc))

            def scal8(t, k, Mx, m0):
                # [Mx, 2, 8] broadcast of per-pair scalar plane over halves
                return tAP(t, 8 * k, [[0, 2], [1, 8]], Mx, m0)

            # ------------- per-pair column phase (3 streams) -------------
            h_ = Mc // 2
            streams = [(0, h_, 0), (h_, Mc - h_, 1)]
            for k, (p, q) in enumerate(pairs):
                d_ = q - p
                PCt = cp.tile([128, NM, 2, 16], f16, tag="PC", name="PC")
                T1t = cp.tile([128, NM, 2, 16], f16, tag="T1", name="T1")
                T2t = cp.tile([128, NM, 2, 16], f16, tag="T2", name="T2")
                for m0, mc, sw in streams:
                    ET = GT if sw == 2 else TT
                    colv = lambda c: aAP(8 * c, [[64, 2], [1, 8]], mc, m0)
                    colJ = lambda c: aAP(8 * c + 64, [[-64, 2], [1, 8]], mc, m0)
                    slot = lambda t, s: tAP(t, 16 * s, [[8, 2], [1, 8]], mc, m0)
                    sel = lambda t, o, st: tAP(t, o, [[st, 2], [1, 8]], mc, m0)
                    ET(slot(T1t, 0), scal8(sr8t, k, mc, m0), colv(q), ALU.mult)
                    ET(slot(T1t, 1), scal8(sr8t, k, mc, m0), colv(p), ALU.mult)
                    ET(slot(T2t, 0), scal8(si8t, k, mc, m0), colJ(q), ALU.mult)
                    ET(slot(T2t, 1), scal8(si8t, k, mc, m0), colJ(p), ALU.mult)
                    ET(slot(PCt, 0), scal8(c8t, k, mc, m0), colv(p), ALU.mult)
                    ET(slot(PCt, 1), scal8(c8t, k, mc, m0), colv(q), ALU.mult)
                    # W+ on sel {(0,re),(1,im)}; W- on sel {(0,im),(1,re)}
                    WT = GT if sw == 1 else ET
                    WT(sel(T1t, 0, 24), sel(T1t, 0, 24), sel(T2t, 0, 24),
                       ALU.add)
                    WT(sel(T1t, 8, 8), sel(T1t, 8, 8), sel(T2t, 8, 8),
                       ALU.subtract)
                    # finals into A columns
                    ET(colv(p), slot(PCt, 0), slot(T1t, 0), ALU.add)
                    ET(colv(q), slot(PCt, 1), slot(T1t, 1), ALU.subtract)
                    # Hermitian row restore (engines per stream)
                    rows_re = aAP(p, [[d_, 2], [8, 8]], mc, m0)
                    cols_re = aAP(8 * p, [[8 * d_, 2], [1, 8]], mc, m0)
                    rows_im = aAP(64 + p, [[d_, 2], [8, 8]], mc, m0)
                    cols_im = aAP(64 + 8 * p, [[8 * d_, 2], [1, 8]], mc, m0)
                    nc.scalar.copy(rows_re, cols_re)
                    nc.scalar.activation(rows_im, cols_im, ACT.Copy,
                                         scale=-1.0)
                # per-pair fixes: diag, im-diag zero, annihilated entries
                nc.gpsimd.tensor_copy(aAP(9 * p, [[1, 1]], Mc),
                                      dAP(p, [[1, 1]], Mc))
                nc.gpsimd.tensor_copy(aAP(9 * q, [[1, 1]], Mc),
                                      dAP(q, [[1, 1]], Mc))
                nc.gpsimd.memset(aAP(64 + 9 * p, [[9 * d_, 2]], Mc), 0.0)
                nc.gpsimd.memset(aAP(8 * p + q, [[7 * d_, 2]], Mc), 0.0)
                nc.gpsimd.memset(aAP(64 + 8 * p + q, [[7 * d_, 2]], Mc), 0.0)

        # ---------------- sweeps + split one-shot correction -------------
        TB = main.tile([128, NM, 8], f32, name="TB")
        nc.vector.memset(TB[:], 0.0)
        Tap = TB[:]
        tpdim = list(Tap.ap[0])

        def tb_at(base, count):
            def tbdst(off, dims):
                return bass.AP(tensor=Tap.tensor,
                               offset=Tap.offset + base * 8 + off,
                               ap=[list(tpdim), [8, count],
                                   *[list(d) for d in dims]])
            return tbdst

        for r in range(1, 8):
            emit_round(r, NM, NM)
        for r in range(1, 6):
            emit_round(r, NM, NM)
        # PT correction reads only frozen A/Dg[32:96]; interleave its rounds
        # with the rho sweep so it fills idle slots
        for r in range(1, 8):
            emit_round(r, NTILES, NTILES)
            emit_params(r, 2 * NTILES, NTILES * 8, NTILES * 128, NM,
                        tbdst=tb_at(NTILES, 2 * NTILES))
        for r in range(1, 8):
            emit_params(r, NTILES, 0, 0, NM, tbdst=tb_at(0, NTILES))

        TT(Dg[:, :, :], Dg[:, :, :], TB[:], ALU.add)

        # ---------------- rho diagonal sort ------------------------------
        tmin = main.tile([128, NTILES], f32, name="tmin")[:]
        dg8 = Dg[:]
        for (i, j) in _CE8:
            di = dg8[:, 0:NTILES, i]
            dj = dg8[:, 0:NTILES, j]
            TT(tmin, di, dj, ALU.min)
            TT(dj, di, dj, ALU.max)
            nc.gpsimd.tensor_copy(di, tmin)

        # ---------------- pt_a / pt_c min & max --------------------------
        mn = main.tile([128, 2 * NTILES], f32, name="mn")[:]
        mx = main.tile([128, 2 * NTILES], f32, name="mx")[:]
        ptd = dg8[:, NTILES:NM, :]
        nc.vector.tensor_reduce(mn, ptd, mybir.AxisListType.X, ALU.min)
        nc.vector.tensor_reduce(mx, ptd, mybir.AxisListType.X, ALU.max)
        mu_min = mn[:, 0:NTILES]
        mu_max = mx[:, 0:NTILES]
        nu_min = mn[:, NTILES:2 * NTILES]
        nu_max = mx[:, NTILES:2 * NTILES]

        # ---------------- loss assembly ----------------------------------
        def L(name):
            return main.tile([128, NTILES], f32, tag=name, name=name)[:]

        w_min = dg8[:, 0:NTILES, 0]
        w_max = dg8[:, 0:NTILES, 7]
        b0, b1_, acc, t1, t2_, t3 = (L("b0"), L("b1"), L("acc"), L("t1"),
                                     L("t2x"), L("t3"))

        nc.vector.tensor_scalar(b0, w_min, -8.0, 1.0, ALU.mult, ALU.add)
        nc.vector.reciprocal(b0, b0)
        nc.vector.tensor_scalar(b1_, w_max, -8.0, 1.0, ALU.mult, ALU.add)
        nc.vector.reciprocal(b1_, b1_)

        assert 1 <= k0 <= 8 and 1 <= k1 <= 8
        nc.gpsimd.tensor_copy(t1, dg8[:, 0:NTILES, 0])
        for i in range(1, k0):
            TT(t1, t1, dg8[:, 0:NTILES, i], ALU.add)
        nc.gpsimd.tensor_copy(t2_, dg8[:, 0:NTILES, 7])
        for i in range(6, 7 - k1, -1):
            TT(t2_, t2_, dg8[:, 0:NTILES, i], ALU.add)
        nc.vector.tensor_scalar(t1, t1, -k0 / 8.0, None, ALU.add)
        TT(t1, t1, b0, ALU.mult)
        nc.vector.tensor_scalar(t2_, t2_, -k1 / 8.0, None, ALU.add)
        TT(t2_, t2_, b1_, ALU.mult)
        TT(t1, t1, t2_, ALU.add)
        nc.vector.tensor_scalar(t1, t1, (k0 + k1) / 8.0, None, ALU.add)
        TT(acc, t1, t1, ALU.mult)
        for beta, ext in ((b0, mu_min), (b1_, mu_max), (b0, nu_min),
                          (b1_, nu_max)):
            nc.vector.tensor_scalar(t3, ext, -0.125, None, ALU.add)
            TT(t3, t3, beta, ALU.mult)
            nc.vector.tensor_scalar(t3, t3, 0.125, None, ALU.add)
            TT(t3, t3, t3, ALU.mult)
            TT(acc, acc, t3, ALU.add)

        nc.sync.dma_start(out=out_d[:, :], in_=acc)

    nc.finalize()
    return nc


_prog_cache = {}


def kernel(rho_vec, rank0, rank1):
    rho_vec = np.asarray(rho_vec, dtype=np.float32)
    k0 = D - int(rank0)
    k1 = D - int(rank1)
    ins = _host_prep(rho_vec)

    from concourse.bass_utils import run_bass_kernel_spmd
    key = (k0, k1)
    if key not in _prog_cache:
        _prog_cache[key] = _build_program(k0, k1)
    nc = _prog_cache[key]
    res = run_bass_kernel_spmd(nc, ins, core_ids=list(range(NCORES)))
    return np.concatenate(
        [np.asarray(res.results[c]["out"]).T.reshape(-1) for c in range(NCORES)]
    ).astype(np.float32)


# revision 21
# speedup vs baseline: 1.0644x; 1.0644x over previous
"""Trainium2 Bass kernel for nn_BESNumEigen3qubitModel (v2).

Same math reduction as v1 (eigenvalues of rho, pt_a(rho), pt_c(rho) per batch
element drive the whole loss), with a faster device algorithm:

  - column-major float layout (f = 64*h + 8*j + i for re/im h of entry (i,j))
    so matrix columns are unit-stride runs -> DVE 2x fp16 mode applies
  - matrices stored fp16, diagonals in a separate f32 tensor
  - 2 full Jacobi sweeps (all 3*32 matrices) + 1 rho-only sweep + a one-shot
    second-order diagonal correction for rho (replaces a 4th sweep)
  - per-round batched rotation-parameter chain (all 4 XOR pairs at once via
    affine access patterns), per-pair column rotations + Hermitian row restore
"""

import numpy as np

D = 8
BATCH = 32768
NCORES = 8
PER_CORE = BATCH // NCORES       # 4096
NTILES = PER_CORE // 128         # 32 rho matrices per partition
NM = 3 * NTILES                  # 96 matrices per partition (type-major)

_f32 = np.float32


# ---------------------------------------------------------------- host prep --

def _gellmann_basis(d):
    mats = []
    for j in range(d):
        for k in range(j + 1, d):
            m = np.zeros((d, d), np.complex128); m[j, k] = 1; m[k, j] = 1
            mats.append(m)
    for j in range(d):
        for k in range(j + 1, d):
            m = np.zeros((d, d), np.complex128); m[j, k] = -1j; m[k, j] = 1j
            mats.append(m)
    for l in range(1, d):
        m = np.zeros((d, d), np.complex128)
        m[np.arange(l), np.arange(l)] = 1
        m[l, l] = -l
        mats.append(np.sqrt(2.0 / (l * (l + 1))) * m)
    return np.stack(mats)


def _pt(m, kind):
    if kind == 'a':
        return np.swapaxes(m.reshape(2, 4, 2, 4), 1, 3).reshape(8, 8)
    return np.swapaxes(m.reshape(4, 2, 4, 2), 1, 3).reshape(8, 8)


def _build_maps():
    """[64, 3*128] f32 map (vec,1) -> col-major floats of rho/pt_a/pt_c,
    and [64, 3*8] diagonal map."""
    G = _gellmann_basis(D)
    basis = list(G) + [np.eye(D) / D]
    M3 = np.zeros((64, 3 * 128), np.float64)
    MD = np.zeros((64, 3 * 8), np.float64)
    for k, A in enumerate(basis):
        for t, At in enumerate((A, _pt(A, 'a'), _pt(A, 'c'))):
            # col-major: f = 64*h + 8*j + i
            M3[k, t * 128:t * 128 + 64] = At.real.T.reshape(-1)
            M3[k, t * 128 + 64:t * 128 + 128] = At.imag.T.reshape(-1)
            MD[k, t * 8:(t + 1) * 8] = np.diagonal(At).real
    return M3.astype(_f32), MD.astype(_f32)


_MAPS = None


def _host_prep(rho_vec):
    global _MAPS
    if _MAPS is None:
        _MAPS = _build_maps()
    M3, MD = _MAPS
    vec = rho_vec.astype(np.float64)
    vec = vec / np.linalg.norm(vec, axis=-1, keepdims=True)
    vec_aug = np.concatenate(
        [vec.astype(_f32), np.ones((vec.shape[0], 1), _f32)], axis=1)
    flat = (vec_aug @ M3).astype(np.float16)               # [B, 384]
    dflat = vec_aug @ MD                                   # [B, 24] f32
    arr = flat.reshape(NCORES, NTILES, 128, 3, 128)
    darr = dflat.reshape(NCORES, NTILES, 128, 3, 8)
    ins = []
    for c in range(NCORES):
        m = np.ascontiguousarray(
            arr[c].transpose(1, 2, 0, 3).reshape(128, NM * 128))
        dg = np.ascontiguousarray(
            darr[c].transpose(1, 2, 0, 3).reshape(128, NM * 8).astype(_f32))
        ins.append({"mats": m, "diag": dg})
    return ins


# ------------------------------------------------------------ device kernel --

def _msb(r):
    return 4 if r >= 4 else (2 if r >= 2 else 1)


# Batcher odd-even mergesort network for 8 elements (19 comparators)
_CE8 = [(0, 1), (2, 3), (4, 5), (6, 7), (0, 2), (1, 3), (4, 6), (5, 7),
        (1, 2), (5, 6), (0, 4), (1, 5), (2, 6), (3, 7), (2, 4), (3, 5),
        (1, 2), (3, 4), (5, 6)]


def _build_program(k0, k1):
    import concourse.bass as bass
    import concourse.bacc as bacc
    import concourse.mybir as mybir
    from concourse.tile import TileContext
    from contextlib import ExitStack

    f32 = mybir.dt.float32
    f16 = mybir.dt.float16
    ALU = mybir.AluOpType
    ACT = mybir.ActivationFunctionType

    nc = bacc.Bacc("TRN2")
    mats_d = nc.dram_tensor("mats", [128, NM * 128], f16, kind="ExternalInput")
    diag_d = nc.dram_tensor("diag", [128, NM * 8], f32, kind="ExternalInput")
    out_d = nc.dram_tensor("out", [128, NTILES], f32, kind="ExternalOutput")

    with ExitStack() as ctx:
        tc = ctx.enter_context(TileContext(nc))
        main = ctx.enter_context(tc.tile_pool(name="main", bufs=1))
        pp = ctx.enter_context(tc.tile_pool(name="pp", bufs=3))
        ep = ctx.enter_context(tc.tile_pool(name="ep", bufs=2))
        cp = ctx.enter_context(tc.tile_pool(name="cp", bufs=2))

        A = main.tile([128, NM, 128], f16, name="A")
        Dg = main.tile([128, NM, 8], f32, name="Dg")
        for ch in range(8):
            nc.sync.dma_start(
                out=A[:, ch * 12:(ch + 1) * 12, :],
                in_=mats_d[:, ch * 12 * 128:(ch + 1) * 12 * 128])
        nc.sync.dma_start(out=Dg[:, :, :], in_=diag_d[:, :])

        eps30 = main.tile([128, 1], f32, name="eps30")
        nc.vector.memset(eps30[:], 1e-30)
        eps35 = main.tile([128, 1], f32, name="eps35")
        nc.vector.memset(eps35[:], 1e-35)

        Aap = A[:]
        pdim = list(Aap.ap[0])
        Dap = Dg[:]
        dpdim = list(Dap.ap[0])

        def aAP(off, dims, M, m0=0):
            return bass.AP(tensor=Aap.tensor,
                           offset=Aap.offset + off + m0 * 128,
                           ap=[list(pdim), [128, M], *[list(d) for d in dims]])

        def dAP(off, dims, M):
            return bass.AP(tensor=Dap.tensor, offset=Dap.offset + off,
                           ap=[list(dpdim), [8, M], *[list(d) for d in dims]])

        def tAP(t, off, dims, M, m0=0):
            # custom inner-dim view of a [128, NM, ...] tile's [m0:m0+M] slice
            tap = t[:]
            return bass.AP(tensor=tap.tensor,
                           offset=tap.offset + off + m0 * tap.ap[1][0],
                           ap=[list(tap.ap[0]), [tap.ap[1][0], M],
                               *[list(d) for d in dims]])

        negone = main.tile([128, 16], f16, name="negone")
        nc.vector.memset(negone[:], -1.0)

        def negbc(Mx):
            nap = negone[:]
            return bass.AP(tensor=nap.tensor, offset=nap.offset,
                           ap=[list(nap.ap[0]), [0, Mx], [0, 2], [1, 8]])

        TT = nc.vector.tensor_tensor
        GT = nc.gpsimd.tensor_tensor
        STT = nc.vector.scalar_tensor_tensor
        GSTT = nc.gpsimd.scalar_tensor_tensor

        # ---------------- one Jacobi round -------------------------------
        def emit_params(r, M, Dv_off, A_off, Mmats, tbdst=None, pre=""):
            """Rotation params for round r over M matrices.

            Returns (csm_t, srp_t, sip_t) tiles [128, Mmats, 4] f32 holding
            per-pair c / sr / si. If tbdst is None, Dg is updated in place
            (dpp/dqq); else tb is accumulated into tbdst AP pair positions.
            """
            hi = _msb(r)
            b1, b2 = [b for b in (1, 2, 4) if b != hi][::-1]  # b1 outer (larger)
            sg_ = lambda b: -b if (r & b) else b
            app = dAP(Dv_off, [[b1, 2], [b2, 2]], M)
            aqq = dAP(Dv_off + r, [[sg_(b1), 2], [sg_(b2), 2]], M)
            sX = [b + 8 * sg_(b) for b in (b1, b2)]
            X = aAP(A_off + 8 * r, [[sX[0], 2], [sX[1], 2]], M)
            Y = aAP(A_off + 8 * r + 64, [[sX[0], 2], [sX[1], 2]], M)

            tiles = {}

            def pt_(tag):
                t = pp.tile([128, Mmats, 4], f32, tag=pre + tag, name=pre + tag)
                tiles[tag] = t
                return (tAP(t, 0, [[2, 2], [1, 2]], M),   # [M, 2, 2] view
                        tAP(t, 0, [[1, 4]], M))           # flat [M, 4] view

            xx, xxf = pt_("xx")
            yy, yyf = pt_("yy")
            m2, m2f = pt_("m2")
            g, gf = pt_("g")
            g2, g2f = pt_("g2")
            s2, s2f = pt_("s2")
            rs, rsf = pt_("rs")
            h, hf = pt_("h")
            ag, agf = pt_("ag")
            den, denf = pt_("den")
            T, Tf = pt_("T")
            sgn, sgnf = pt_("sgn")
            v, vf = pt_("v")
            tb, tbf = pt_("tb")

            lat = True
            MT = TT if lat else GT
            nc.scalar.activation(xx, X, ACT.Square)
            nc.scalar.activation(yy, Y, ACT.Square)
            MT(m2f, xxf, yyf, ALU.add)
            GT(g, app, aqq, ALU.subtract)
            nc.scalar.activation(g2f, gf, ACT.Square)
            if lat:
                STT(s2f, m2f, 4.0, g2f, ALU.mult, ALU.add)
            else:
                GT(s2f, m2f, m2f, ALU.add)
                GT(s2f, s2f, s2f, ALU.add)
                GT(s2f, s2f, g2f, ALU.add)
            nc.scalar.activation(rsf, s2f, ACT.Abs_reciprocal_sqrt,
                                 bias=eps30[:])
            nc.scalar.activation(agf, gf, ACT.Abs)
            MT(hf, s2f, rsf, ALU.mult)
            MT(denf, agf, hf, ALU.add)
            if lat:
                nc.vector.reciprocal(Tf, denf)
            else:
                GT(Tf, denf, denf, ALU.mult)
                nc.scalar.activation(Tf, Tf, ACT.Abs_reciprocal_sqrt,
                                     bias=eps30[:])   # T = 1/den
            nc.scalar.sign(sgnf, gf, bias=eps35[:])
            GT(vf, m2f, Tf, ALU.mult)
            GT(vf, vf, vf, ALU.add)              # v = 2*m2*T
            GT(tbf, vf, sgnf, ALU.mult)          # tb = 2*m2*T*sg (full)

            if tbdst is None:
                GT(app, app, tb, ALU.add)        # Dg[p] += tb
                GT(aqq, aqq, tb, ALU.subtract)   # Dg[q] -= tb
                T2, T2f = pt_("T2")
                t2, t2f = pt_("t2")
                csm, csmf = pt_("csm")
                u2, u2f = pt_("u2")
                urb2, urb2f = pt_("urb2")
                srp, srpf = pt_("srp")
                sip, sipf = pt_("sip")
                nc.scalar.activation(T2f, Tf, ACT.Square)
                STT(t2f, T2f, 4.0, m2f, ALU.mult, ALU.mult)
                nc.scalar.activation(csmf, t2f, ACT.Abs_reciprocal_sqrt,
                                     bias=1.0)
                GT(u2f, Tf, sgnf, ALU.mult)
                STT(urb2f, u2f, 2.0, csmf, ALU.mult, ALU.mult)
                TT(srp, urb2, X, ALU.mult)
                TT(sip, urb2, Y, ALU.mult)
                return tiles["csm"], tiles["srp"], tiles["sip"]
            else:
                tbp = tbdst(0, [[b1, 2], [b2, 2]])
                tbq = tbdst(r, [[sg_(b1), 2], [sg_(b2), 2]])
                GT(tbp, tbp, tb, ALU.add)
                GT(tbq, tbq, tb, ALU.subtract)
                return None

        def emit_round(r, M, Mc):
            """Round r: params over M matrices, col updates over Mc."""
            hi = _msb(r)
            b1, b2 = [b for b in (1, 2, 4) if b != hi][::-1]
            sg_ = lambda b: -b if (r & b) else b
            pairs = [(a, a ^ r) for a in (0, b2, b1, b1 + b2)]

            csm_t, srp_t, sip_t = emit_params(r, M, 0, 0, NM)

            # expansions over the 8-run (packed last dim for DVE 2x)
            c8t = ep.tile([128, NM, 4, 8], f16, tag="c8", name="c8")
            sr8t = ep.tile([128, NM, 4, 8], f16, tag="sr8", name="sr8")
            si8t = ep.tile([128, NM, 4, 8], f16, tag="si8", name="si8")

            def bc8(t, Mx):
                return tAP(t, 0, [[1, 4], [0, 8]], Mx)

            nc.scalar.copy(c8t[:][:, 0:Mc], bc8(csm_t, Mc))
            nc.vector.tensor_copy(sr8t[:][:, 0:Mc], bc8(srp_t, Mc))
            nc.scalar.copy(si8t[:][:, 0:Mc], bc8(sip_t, Mc))

            def scal8(t, k, Mx, m0):
                # [Mx, 2, 8] broadcast of per-pair scalar plane over halves
                return tAP(t, 8 * k, [[0, 2], [1, 8]], Mx, m0)

            # ------------- per-pair column phase (3 streams) -------------
            h_ = Mc // 2
            streams = [(0, h_, 0), (h_, Mc - h_, 1)]
            for k, (p, q) in enumerate(pairs):
                d_ = q - p
                PCt = cp.tile([128, NM, 2, 16], f16, tag="PC", name="PC")
                T1t = cp.tile([128, NM, 2, 16], f16, tag="T1", name="T1")
                T2t = cp.tile([128, NM, 2, 16], f16, tag="T2", name="T2")
                for m0, mc, sw in streams:
                    ET = GT if sw == 2 else TT
                    colv = lambda c: aAP(8 * c, [[64, 2], [1, 8]], mc, m0)
                    colJ = lambda c: aAP(8 * c + 64, [[-64, 2], [1, 8]], mc, m0)
                    slot = lambda t, s: tAP(t, 16 * s, [[8, 2], [1, 8]], mc, m0)
                    sel = lambda t, o, st: tAP(t, o, [[st, 2], [1, 8]], mc, m0)
                    ET(slot(T1t, 0), scal8(sr8t, k, mc, m0), colv(q), ALU.mult)
                    ET(slot(T1t, 1), scal8(sr8t, k, mc, m0), colv(p), ALU.mult)
                    ET(slot(T2t, 0), scal8(si8t, k, mc, m0), colJ(q), ALU.mult)
                    ET(slot(T2t, 1), scal8(si8t, k, mc, m0), colJ(p), ALU.mult)
                    ET(slot(PCt, 0), scal8(c8t, k, mc, m0), colv(p), ALU.mult)
                    ET(slot(PCt, 1), scal8(c8t, k, mc, m0), colv(q), ALU.mult)
                    # W+ on sel {(0,re),(1,im)}; W- on sel {(0,im),(1,re)}
                    ET(sel(T1t, 0, 24), sel(T1t, 0, 24), sel(T2t, 0, 24),
                       ALU.add)
                    ET(sel(T1t, 8, 8), sel(T1t, 8, 8), sel(T2t, 8, 8),
                       ALU.subtract)
                    # finals into A columns
                    ET(colv(p), slot(PCt, 0), slot(T1t, 0), ALU.add)
                    ET(colv(q), slot(PCt, 1), slot(T1t, 1), ALU.subtract)
                    # Hermitian row restore (engines per stream)
                    rows_re = aAP(p, [[d_, 2], [8, 8]], mc, m0)
                    cols_re = aAP(8 * p, [[8 * d_, 2], [1, 8]], mc, m0)
                    rows_im = aAP(64 + p, [[d_, 2], [8, 8]], mc, m0)
                    cols_im = aAP(64 + 8 * p, [[8 * d_, 2], [1, 8]], mc, m0)
                    nc.scalar.copy(rows_re, cols_re)
                    nc.scalar.activation(rows_im, cols_im, ACT.Copy,
                                         scale=-1.0)
                # per-pair fixes: diag, im-diag zero, annihilated entries
                nc.gpsimd.tensor_copy(aAP(9 * p, [[1, 1]], Mc),
                                      dAP(p, [[1, 1]], Mc))
                nc.gpsimd.tensor_copy(aAP(9 * q, [[1, 1]], Mc),
                                      dAP(q, [[1, 1]], Mc))
                nc.gpsimd.memset(aAP(64 + 9 * p, [[9 * d_, 2]], Mc), 0.0)
                nc.gpsimd.memset(aAP(8 * p + q, [[7 * d_, 2]], Mc), 0.0)
                nc.gpsimd.memset(aAP(64 + 8 * p + q, [[7 * d_, 2]], Mc), 0.0)

        # ---------------- sweeps + split one-shot correction -------------
        TB = main.tile([128, NM, 8], f32, name="TB")
        nc.vector.memset(TB[:], 0.0)
        Tap = TB[:]
        tpdim = list(Tap.ap[0])

        def tb_at(base, count):
            def tbdst(off, dims):
                return bass.AP(tensor=Tap.tensor,
                               offset=Tap.offset + base * 8 + off,
                               ap=[list(tpdim), [8, count],
                                   *[list(d) for d in dims]])
            return tbdst

        for r in range(1, 8):
            emit_round(r, NM, NM)
        for r in range(1, 6):
            emit_round(r, NM, NM)
        # PT correction reads only frozen A/Dg[32:96]; interleave its rounds
        # with the rho sweep so it fills idle slots
        for r in range(1, 8):
            emit_round(r, NTILES, NTILES)
            emit_params(r, 2 * NTILES, NTILES * 8, NTILES * 128, NM,
                        tbdst=tb_at(NTILES, 2 * NTILES))
        for r in range(1, 8):
            emit_params(r, NTILES, 0, 0, NM, tbdst=tb_at(0, NTILES))

        TT(Dg[:, :, :], Dg[:, :, :], TB[:], ALU.add)

        # ---------------- rho diagonal sort ------------------------------
        tmin = main.tile([128, NTILES], f32, name="tmin")[:]
        dg8 = Dg[:]
        for (i, j) in _CE8:
            di = dg8[:, 0:NTILES, i]
            dj = dg8[:, 0:NTILES, j]
            TT(tmin, di, dj, ALU.min)
            TT(dj, di, dj, ALU.max)
            nc.gpsimd.tensor_copy(di, tmin)

        # ---------------- pt_a / pt_c min & max --------------------------
        mn = main.tile([128, 2 * NTILES], f32, name="mn")[:]
        mx = main.tile([128, 2 * NTILES], f32, name="mx")[:]
        ptd = dg8[:, NTILES:NM, :]
        nc.vector.tensor_reduce(mn, ptd, mybir.AxisListType.X, ALU.min)
        nc.vector.tensor_reduce(mx, ptd, mybir.AxisListType.X, ALU.max)
        mu_min = mn[:, 0:NTILES]
        mu_max = mx[:, 0:NTILES]
        nu_min = mn[:, NTILES:2 * NTILES]
        nu_max = mx[:, NTILES:2 * NTILES]

        # ---------------- loss assembly ----------------------------------
        def L(name):
            return main.tile([128, NTILES], f32, tag=name, name=name)[:]

        w_min = dg8[:, 0:NTILES, 0]
        w_max = dg8[:, 0:NTILES, 7]
        b0, b1_, acc, t1, t2_, t3 = (L("b0"), L("b1"), L("acc"), L("t1"),
                                     L("t2x"), L("t3"))

        nc.vector.tensor_scalar(b0, w_min, -8.0, 1.0, ALU.mult, ALU.add)
        nc.vector.reciprocal(b0, b0)
        nc.vector.tensor_scalar(b1_, w_max, -8.0, 1.0, ALU.mult, ALU.add)
        nc.vector.reciprocal(b1_, b1_)

        assert 1 <= k0 <= 8 and 1 <= k1 <= 8
        nc.gpsimd.tensor_copy(t1, dg8[:, 0:NTILES, 0])
        for i in range(1, k0):
            TT(t1, t1, dg8[:, 0:NTILES, i], ALU.add)
        nc.gpsimd.tensor_copy(t2_, dg8[:, 0:NTILES, 7])
        for i in range(6, 7 - k1, -1):
            TT(t2_, t2_, dg8[:, 0:NTILES, i], ALU.add)
        nc.vector.tensor_scalar(t1, t1, -k0 / 8.0, None, ALU.add)
        TT(t1, t1, b0, ALU.mult)
        nc.vector.tensor_scalar(t2_, t2_, -k1 / 8.0, None, ALU.add)
        TT(t2_, t2_, b1_, ALU.mult)
        TT(t1, t1, t2_, ALU.add)
        nc.vector.tensor_scalar(t1, t1, (k0 + k1) / 8.0, None, ALU.add)
        TT(acc, t1, t1, ALU.mult)
        for beta, ext in ((b0, mu_min), (b1_, mu_max), (b0, nu_min),
                          (b1_, nu_max)):
            nc.vector.tensor_scalar(t3, ext, -0.125, None, ALU.add)
            TT(t3, t3, beta, ALU.mult)
            nc.vector.tensor_scalar(t3, t3, 0.125, None, ALU.add)
            TT(t3, t3, t3, ALU.mult)
            TT(acc, acc, t3, ALU.add)

        nc.sync.dma_start(out=out_d[:, :], in_=acc)

    nc.finalize()
    return nc


_prog_cache = {}


def kernel(rho_vec, rank0, rank1):
    rho_vec = np.asarray(rho_vec, dtype=np.float32)
    k0 = D - int(rank0)
    k1 = D - int(rank1)
    ins = _host_prep(rho_vec)

    from concourse.bass_utils import run_bass_kernel_spmd
    key = (k0, k1)
    if key not in _prog_cache:
        _prog_cache[key] = _build_program(k0, k1)
    nc = _prog_cache[key]
    res = run_bass_kernel_spmd(nc, ins, core_ids=list(range(NCORES)))
    return np.concatenate(
        [np.asarray(res.results[c]["out"]).T.reshape(-1) for c in range(NCORES)]
    ).astype(np.float32)
